# revision 9
# baseline (speedup 1.0000x reference)
"""Causal self-attention (B=2, S=2048, E=1024, H=16) on 8 trn2 cores.

Sharding: batch x head -- core c handles batch c//4 and the 4 heads
starting at (c%4)*4. Each core runs QKV projection for its heads,
causal attention, and its slice of the output projection (row-split
c_proj); the host sums the 4 partial projections per batch.

v2 design notes:
- All matmul operands fp16 (1 PE cycle/row at any moving size); inputs
  are cast to fp16 on the host, halving input DMA.
- Phases are fused into one PE stream: qkv(0), qkv(1), attn(0),
  qkv(2), attn(1), qkv(3), attn(2), proj(0), attn(3), proj(1..3).
- Attention inner loop is software-pipelined with skew 1: the PE
  computes scores for chunk i+1 while ACT exps chunk i, so exp never
  stalls the PE (and the PE stays at full pstate).
- Scores are computed transposed (S^T[k, q]) per head with a
  two-head-packed zero-padded K so every matmul is a full 128-row
  tile; AV rides a ones-column in V to produce rowsums for free.
- Softmax normalization: DVE reciprocal of the rowsum row (PSUM),
  DMA partition-broadcast of the recip row, one DVE multiply into the
  fp16 proj stationary. No PE or ACT involvement.
"""

import os
import sys

import numpy as np

_DIR = os.path.dirname(os.path.abspath(__file__))
for _p in (_DIR,):
    if _p not in sys.path:
        sys.path.insert(0, _p)

import concourse.bass as bass
import concourse.mybir as mybir
from concourse import tile
from concourse.vector_clock import ScopedClock, VectorClock

F32 = mybir.dt.float32
F32R = mybir.dt.float32r
F16 = mybir.dt.float16
U16 = mybir.dt.uint16

B, S, E, H, D = 2, 2048, 1024, 16, 64
HPC = 4          # heads per core
N_CORES = 8
QT = 512         # q tile (moving dim)
KC = 128         # k chunk (contraction tile)
EC = E // 128    # 8 contraction chunks over the embedding dim
NQ = S // QT     # 4 q tiles
NST = S // KC    # 16 kv tiles of 128


class SplitDrainTileContext(tile.TileContext):
    """Kernel-tail drain with its sem waits split one per instruction.

    The walrus build here rejects instructions carrying more sync waits
    than their ISA struct encodes; TileContext hangs one wait per live
    proc on a single Drain. Sequential single-wait drains on the sync
    engine give the same guarantee.
    """

    def _drain_and_barrier(self, tick_clock, wait_clock):
        gc = list(tick_clock.global_clock)
        n = len(gc)
        for i, t in enumerate(gc):
            if t:
                vc = VectorClock([t if j == i else 0 for j in range(n)])
                inst = self.nc.sync.drain()
                wait_clock.add_sem_waits(inst.ins, ScopedClock({None: vc}))
        self.nc.all_engine_barrier()
        assert self.sems is not None
        popped = self.nc._tile_sem_poison_stack.pop()
        assert popped is self._sem_poison
        self.nc.clear_and_free_semaphores(list(self.sems.allocated().values()))
        self.nc.all_engine_barrier()


# ---------------------------------------------------------------- BIR fix

_CAPS = {"EventSemaphore": 2}
_DEFAULT_CAP = 1
_counter = [0]


def _split_bir_waits(bir):
    """Move excess sync waits onto EventSemaphores inserted just before
    the overloaded instruction (same engine => same program order)."""
    n = 0
    for fn in bir.get("functions", []):
        for bb in fn.get("blocks", []):
            out = []
            for inst in bb.get("instructions", []):
                si = inst.get("sync_info")
                waits = si.get("on_wait") if si else None
                cap = _CAPS.get(inst.get("opcode"), _DEFAULT_CAP)
                if waits and len(waits) > cap:
                    excess, keep = waits[:-cap], waits[-cap:]
                    for i in range(0, len(excess), 2):
                        _counter[0] += 1
                        out.append({
                            "debug": inst.get("debug", 0),
                            "engine": inst["engine"],
                            "ins": [], "outs": [],
                            "name": f"antsplitw-{_counter[0]}",
                            "opcode": "EventSemaphore",
                            "sync_info": {"on_update": [],
                                          "on_wait": excess[i:i + 2]},
                        })
                        n += 1
                    si["on_wait"] = keep
                out.append(inst)
            bb["instructions"] = out
    return n


def _install_bir_fix():
    import json
    import concourse.bass2jax as bass2jax
    from concourse.bass_utils import compile_bir_kernel as orig
    if getattr(bass2jax.compile_bir_kernel, "_ant_split", False):
        return

    def wrapped(ant_bir_str, *args, **kwargs):
        bir = json.loads(ant_bir_str)
        if _split_bir_waits(bir):
            ant_bir_str = json.dumps(bir).encode()
        return orig(ant_bir_str, *args, **kwargs)

    wrapped._ant_split = True
    bass2jax.compile_bir_kernel = wrapped


# ---------------------------------------------------------------- device

def build():
    nc = bass.Bass("TRN2", target_bir_lowering=False, debug=False)
    xT_d = nc.dram_tensor("xT", [E, S], F16, kind="ExternalInput").ap()
    wqk_d = nc.dram_tensor("wqk", [E, 2 * HPC * D], F16, kind="ExternalInput").ap()
    wv_d = nc.dram_tensor("wv", [E, HPC * D], F16, kind="ExternalInput").ap()
    wp_d = nc.dram_tensor("wproj", [HPC * D, E], F16, kind="ExternalInput").ap()
    y_d = nc.dram_tensor("y", [S, E], F16, kind="ExternalOutput").ap()

    EXP = mybir.ActivationFunctionType.Exp

    with SplitDrainTileContext(nc) as tc:
        with tc.tile_pool(name="persist", bufs=1) as persist:
            xT_sb = persist.tile([128, EC, S], F16)
            wqk_sb = persist.tile([128, EC, 512], F16)
            wv_sb = persist.tile([128, EC, 256], F16)
            wp_sb = persist.tile([128, 2, E], F16)
            qT = persist.tile([128, 2, S], F16)      # heads 01 | 23 stacked
            kTpad = persist.tile([128, HPC, S], F16)  # per head, half rows zero
            vaug = persist.tile([128, NST, HPC, D + 1], F16)
            yT = persist.tile([128, 2, S], F16)       # normalized, proj lhsT
            ones = persist.tile([128, 64], F32R)

            # dead halves of kTpad zeroed once; ones column of vaug
            nc.vector.memset(ones[:].bitcast(F32), 1.0)
            nc.vector.memset(vaug[:, :, :, D:D + 1].bitcast(U16), 15360)
            for h in range(HPC):
                dead = slice(64, 128) if h % 2 == 0 else slice(0, 64)
                eng = nc.gpsimd if h < 2 else nc.vector
                eng.memset(kTpad[dead, h, :].bitcast(U16), 0)

            # ---- input DMA: first-needed first, split across queues ----
            for ec in range(EC):
                nc.scalar.dma_start(xT_sb[:, ec, 0:QT],
                                    xT_d[ec * 128:(ec + 1) * 128, 0:QT])
                nc.sync.dma_start(wqk_sb[:, ec, :],
                                  wqk_d[ec * 128:(ec + 1) * 128, :])
            for ec in range(EC):
                nc.sync.dma_start(xT_sb[:, ec, QT:2 * QT],
                                  xT_d[ec * 128:(ec + 1) * 128, QT:2 * QT])
                nc.gpsimd.dma_start(wv_sb[:, ec, :],
                                    wv_d[ec * 128:(ec + 1) * 128, :])
            for ec in range(EC):
                nc.gpsimd.dma_start(xT_sb[:, ec, 2 * QT:3 * QT],
                                    xT_d[ec * 128:(ec + 1) * 128, 2 * QT:3 * QT])
                nc.sync.dma_start(xT_sb[:, ec, 3 * QT:4 * QT],
                                  xT_d[ec * 128:(ec + 1) * 128, 3 * QT:4 * QT])
            for ci in range(2):
                nc.scalar.dma_start(wp_sb[:, ci, :],
                                    wp_d[ci * 128:(ci + 1) * 128, :])

            with (
                tc.tile_pool(name="ps1", bufs=2, space="PSUM") as ps1,
                tc.tile_pool(name="pss", bufs=3, space="PSUM") as pss,
                tc.tile_pool(name="psav", bufs=3, space="PSUM") as psav,
                tc.tile_pool(name="ptp", bufs=6) as ptp,
                tc.tile_pool(name="nrm", bufs=4) as nrm,
                tc.tile_pool(name="bcs", bufs=4) as bcs,
                tc.tile_pool(name="pout", bufs=3) as pout,
            ):
                def qk_block(q4, on_act):
                    sslc = slice(q4 * QT, (q4 + 1) * QT)
                    for rt in range(4):
                        ps = ps1.tile([128, 512], F32, tag="ps1")
                        for ec in range(EC):
                            nc.tensor.matmul(
                                ps[:],
                                wqk_sb[:, ec, rt * 128:(rt + 1) * 128],
                                xT_sb[:, ec, sslc],
                                start=(ec == 0), stop=(ec == EC - 1))
                        if rt < 2:
                            if on_act:
                                nc.scalar.copy(qT[:, rt, sslc], ps[:])
                            else:
                                nc.vector.tensor_copy(out=qT[:, rt, sslc],
                                                      in_=ps[:])
                        else:
                            h0 = 2 * (rt - 2)
                            if on_act:
                                nc.scalar.copy(kTpad[0:64, h0, sslc],
                                               ps[0:64, :])
                                nc.scalar.copy(kTpad[64:128, h0 + 1, sslc],
                                               ps[64:128, :])
                            else:
                                nc.vector.tensor_copy(
                                    out=kTpad[0:64, h0, sslc], in_=ps[0:64, :])
                                nc.vector.tensor_copy(
                                    out=kTpad[64:128, h0 + 1, sslc],
                                    in_=ps[64:128, :])

                def v_block(q4):
                    for st2 in range(4 * q4, 4 * q4 + 4):
                        ps = ps1.tile([128, 512], F32, tag="ps1")
                        for ec in range(EC):
                            nc.tensor.matmul(
                                ps[:, 0:256],
                                xT_sb[:, ec, st2 * 128:(st2 + 1) * 128],
                                wv_sb[:, ec, :],
                                start=(ec == 0), stop=(ec == EC - 1))
                        nc.vector.tensor_copy(
                            out=vaug[:, st2, :, 0:D],
                            in_=ps[:, 0:256].rearrange("p (h d) -> p h d",
                                                       h=HPC))

                def nm_head(qj, h, avt):
                    # rowsum row -> SBUF f32r -> PE broadcast -> 1/x -> scale
                    rs = nrm.tile([1, QT], F32R, tag="rs")
                    with nc.allow_low_precision(reason="rowsum f32r"):
                        nc.vector.tensor_copy(out=rs[0:1, :],
                                              in_=avt[64:65, :])
                    bc = ps1.tile([128, QT], F32, tag="ps1", name="bc")
                    nc.tensor.matmul(bc[0:64, :], ones[0:1, 0:64], rs[:],
                                     start=True, stop=True)
                    bc_sb = bcs.tile([64, QT], F32, tag="bc")
                    nc.vector.reciprocal(bc_sb[:], bc[0:64, :])
                    po = 64 * (h % 2)
                    with nc.allow_low_precision(reason="proj lhsT fp16"):
                        nc.vector.tensor_mul(
                            yT[po:po + 64, h // 2, qj * QT:(qj + 1) * QT],
                            avt[0:64, :], bc_sb[:, :])

                def attn_block(qj):
                    nkc = (qj + 1) * QT // KC
                    chunks = [(h, kc) for h in range(HPC) for kc in range(nkc)]
                    av_tiles = {}
                    pending = []

                    def do_S(h, kc):
                        o = max(0, kc * KC - qj * QT)
                        ps = pss.tile([128, QT], F32, tag="pss")
                        nc.tensor.matmul(
                            ps[:, o:QT],
                            kTpad[:, h, kc * KC:(kc + 1) * KC],
                            qT[:, h // 2, qj * QT + o:(qj + 1) * QT],
                            start=True, stop=True)
                        pt = ptp.tile([128, QT], F16, tag="pt")
                        nc.scalar.activation(pt[:, o:QT], ps[:, o:QT],
                                             EXP, scale=0.125)
                        if kc * KC >= qj * QT:
                            w = min(KC, QT - o)
                            nc.gpsimd.affine_select(
                                out=pt[:, o:o + w], in_=pt[:, o:o + w],
                                compare_op=mybir.AluOpType.is_ge,
                                fill=0.0, base=qj * QT + o - kc * KC,
                                pattern=[[1, w]], channel_multiplier=-1)
                        return (h, kc, o, pt)

                    def do_AV(h, kc, o, pt):
                        if kc == 0:
                            av_tiles[h] = psav.tile([65, QT], F32, tag="av",
                                                    name="av")
                        nc.tensor.matmul(av_tiles[h][:, o:QT],
                                         vaug[:, kc, h, :],
                                         pt[:, o:QT],
                                         start=(kc == 0),
                                         stop=(kc == nkc - 1))
                        if kc == nkc - 1:
                            nm_head(qj, h, av_tiles[h])

                    for item in chunks:
                        pending.append(do_S(*item))
                        if len(pending) > 1:
                            do_AV(*pending.pop(0))
                    while pending:
                        do_AV(*pending.pop(0))

                def proj_block(qj, on_act, dma_eng):
                    for qt in range(4 * qj, 4 * qj + 4):
                        for eo in range(2):
                            pp = ps1.tile([128, 512], F32, tag="ps1")
                            for ci in range(2):
                                nc.tensor.matmul(
                                    pp[:],
                                    yT[:, ci, qt * 128:(qt + 1) * 128],
                                    wp_sb[:, ci, eo * 512:(eo + 1) * 512],
                                    start=(ci == 0), stop=(ci == 1))
                            po_t = pout.tile([128, 512], F16)
                            if on_act:
                                nc.scalar.copy(po_t[:], pp[:])
                            else:
                                nc.vector.tensor_copy(out=po_t[:], in_=pp[:])
                            dma_eng.dma_start(
                                y_d[qt * 128:(qt + 1) * 128,
                                    eo * 512:(eo + 1) * 512],
                                po_t[:])

                qk_block(0, on_act=True)
                v_block(0)
                qk_block(1, on_act=True)
                v_block(1)
                attn_block(0)
                qk_block(2, on_act=False)
                v_block(2)
                attn_block(1)
                qk_block(3, on_act=False)
                v_block(3)
                attn_block(2)
                proj_block(0, on_act=False, dma_eng=nc.gpsimd)
                attn_block(3)
                proj_block(1, on_act=False, dma_eng=nc.gpsimd)
                proj_block(2, on_act=True, dma_eng=nc.sync)
                proj_block(3, on_act=True, dma_eng=nc.sync)
    return nc


# ---------------------------------------------------------------- host

_NC_CACHE = []


def _get_nc():
    if not _NC_CACHE:
        _install_bir_fix()
        _NC_CACHE.append(build())
    return _NC_CACHE[0]


def make_in_maps(x, w_attn, w_proj):
    in_maps = []
    for c in range(N_CORES):
        b, h0 = c // 4, (c % 4) * HPC
        wq = w_attn[:, h0 * D:(h0 + HPC) * D]
        wk = w_attn[:, E + h0 * D:E + (h0 + HPC) * D]
        wv = w_attn[:, 2 * E + h0 * D:2 * E + (h0 + HPC) * D]
        in_maps.append({
            "xT": np.ascontiguousarray(x[b].T.astype(np.float16)),
            "wqk": np.ascontiguousarray(
                np.concatenate([wq, wk], axis=1).astype(np.float16)),
            "wv": np.ascontiguousarray(wv.astype(np.float16)),
            "wproj": np.ascontiguousarray(
                w_proj[h0 * D:(h0 + HPC) * D, :].astype(np.float16)),
        })
    return in_maps


def run(x, w_attn, w_proj, trace=False, tmpdir=None):
    from concourse.bass_utils import run_bass_kernel_spmd
    nc = _get_nc()
    res = run_bass_kernel_spmd(nc, make_in_maps(x, w_attn, w_proj),
                               list(range(N_CORES)), trace=trace, tmpdir=tmpdir)
    y = np.zeros((B, S, E), np.float32)
    for c in range(N_CORES):
        y[c // 4] += res.results[c]["y"].astype(np.float32)
    return y, res


def kernel(x, w_attn, w_proj):
    y, _ = run(np.asarray(x, np.float32), np.asarray(w_attn, np.float32),
               np.asarray(w_proj, np.float32))
    return y
